# revision 1
# baseline (speedup 1.0000x reference)
"""ConvLSTM3D Trainium2 kernel.

Data-parallel over batch: 8 batch elements -> 8 NeuronCores, weights
replicated.  Per core, each timestep computes a 3x3x3 conv over
concat(x_t, H) via PSUM-accumulated bf16 matmuls (one matmul per conv
tap; H taps run two-at-a-time in the 128x128 PE array via row tiling),
then the LSTM gate math on DVE/ACT in fp32.

Layouts (per core):
  - H state lives in SBUF as a zero-padded bf16 volume HH[128, 18*34*34]:
    rows 0-63 hold H_pad, rows 64-127 hold H_pad shifted one padded column
    left (upper[p] = lower[p+1]).  A single K=128 matmul reading both
    halves at the kw=-1 offset therefore contracts taps kw=-1 AND kw=0 at
    once; only kw=+1 needs its own (zero-padded to K=128) matmul.
  - X is fed pre-padded/bf16 from HBM; an im2col buffer XI[108, ...] with
    all 27 taps stacked on partitions handles the whole x-contribution in
    one matmul per output tile.
  - C state is fp32 [128, NSP] (both halves hold C) so the i/f gate math
    runs 128 partitions wide.
"""
import numpy as np
import ml_dtypes
from contextlib import ExitStack
from itertools import product

import concourse.bacc as bacc
import concourse.bass as bass
import concourse.tile as tile
import concourse.mybir as mybir

f32 = mybir.dt.float32
bf16 = mybir.dt.bfloat16

# Problem constants (hardcoded per harness contract)
B, C_IN, T, D, HS, WS = 8, 4, 8, 16, 32, 32
C_OUT = 64
PD, PH, PW = D + 2, HS + 2, WS + 2          # 18, 34, 34
PLANE = PH * PW                              # 1156
NPAD = PD * PLANE                            # 20808
NSP = D * HS * WS                            # 16384
QOFF = PLANE + PW + 1                        # 1191, min padded read index
XI_ROWS = 543                                # ceil((NPAD - 2*QOFF)/34) aligned
XI_COLS = XI_ROWS * PW                       # 18462

TAPS = [(kd, kh, kw) for kd, kh, kw in product((-1, 0, 1), repeat=3)]  # 27
KDKH = [(kd, kh) for kd, kh in product((-1, 0, 1), repeat=2)]          # 9
N_WSLOT = 19  # 9 fused (kw=-1,0) + 9 single (kw=+1) + 1 x-im2col


def build_nc(t_steps=T, d_depth=D, dbg_t=None):
    """Build the Bass program. Reduced (t_steps, d_depth) for sim tests
    still uses the full padded plane geometry, just fewer planes/steps."""
    nsp = d_depth * HS * WS
    npad = (d_depth + 2) * PLANE
    plane = PLANE
    xi_rows = 34 * (d_depth - 1) + 16 + 16  # last r0 + 16
    xi_cols = xi_rows * PW

    nc = bacc.Bacc("TRN2", target_bir_lowering=False, debug=False)

    xpad_h = nc.dram_tensor("xpad", [C_IN, t_steps, npad], bf16, kind="ExternalInput")
    whwx_h = nc.dram_tensor("whwx", [128, N_WSLOT * 256], bf16, kind="ExternalInput")
    bias_h = nc.dram_tensor("bias", [128, 4], f32, kind="ExternalInput")
    ppif_h = nc.dram_tensor("ppif", [128, nsp], bf16, kind="ExternalInput")
    ppo_h = nc.dram_tensor("ppo", [64, nsp], bf16, kind="ExternalInput")
    y_h = nc.dram_tensor("y", [C_OUT, t_steps * nsp], f32, kind="ExternalOutput")
    dbg_h = None
    if dbg_t is not None:
        dbg_h = nc.dram_tensor("dbg", [128, nsp], f32, kind="ExternalOutput")

    with tile.TileContext(nc) as tc, ExitStack() as ctx:
        const = ctx.enter_context(tc.tile_pool(name="const", bufs=1))
        pp_pool = ctx.enter_context(tc.tile_pool(name="pp", bufs=2))
        sc = ctx.enter_context(tc.tile_pool(name="sc", bufs=2))
        mosc = ctx.enter_context(tc.tile_pool(name="mosc", bufs=2))
        psum = ctx.enter_context(tc.tile_pool(name="psum", bufs=2, space="PSUM"))

        HH = const.tile([128, npad], bf16)
        XI = const.tile([128, xi_cols], bf16)
        C2 = const.tile([128, nsp], f32)
        WW = const.tile([128, N_WSLOT * 256], bf16)
        BIAS = const.tile([128, 4], f32)

        nc.vector.memset(HH[:], 0.0)
        nc.vector.memset(XI[:], 0.0)
        nc.vector.memset(C2[:], 0.0)
        nc.sync.dma_start(WW[:], whwx_h[:])
        nc.sync.dma_start(BIAS[:], bias_h[:])

        hh3 = HH[:].rearrange("p (r c) -> p r c", c=PW)
        xi3 = XI[:].rearrange("p (r c) -> p r c", c=PW)
        ww3 = WW[:].rearrange("p (s m) -> p s m", m=256)

        b_if = BIAS[:, 0:1]
        b_c = BIAS[0:64, 1:2]
        b_o = BIAS[0:64, 2:3]
        b_zero = BIAS[0:64, 3:4]  # host-provided zeros (avoid const_aps)

        def emit_mm(t, d):
            """Conv matmuls for chunk (t, d) -> (p0, p1) PSUM tiles."""
            p0 = psum.tile([128, 1024], f32, tag="p0")
            p1 = psum.tile([128, 1024], f32, tag="p1")
            for mt, ptile in ((0, p0), (1, p1)):
                msl = slice(mt * 128, (mt + 1) * 128)
                for hb in (0, 1):
                    out_ap = ptile[:, hb * 512:(hb + 1) * 512]
                    first = True
                    if t > 0:
                        for s, (kd, kh) in enumerate(KDKH):
                            r0 = (d + 1 + kd) * 34 + hb * 16 + 1 + kh
                            # fused: lower=tap kw=-1, upper(shifted)=tap kw=0
                            nc.tensor.matmul(
                                out_ap, ww3[:, s, msl],
                                hh3[:, r0:r0 + 16, 0:32],
                                start=first, stop=False, skip_group_check=True,
                            )
                            first = False
                            # single: tap kw=+1 (upper lhsT rows are zero)
                            nc.tensor.matmul(
                                out_ap, ww3[:, 9 + s, msl],
                                hh3[:, r0:r0 + 16, 2:34],
                                start=False, stop=False, skip_group_check=True,
                            )
                    r0x = 34 * d + hb * 16
                    nc.tensor.matmul(
                        out_ap, ww3[:, 18, msl],
                        xi3[:, r0x:r0x + 16, 0:32],
                        start=first, stop=True, skip_group_check=True,
                    )
            return p0, p1

        def emit_el(t, d, p0, p1):
            """Gate math + state/output writes for chunk (t, d)."""
            csl = slice(d * 1024, (d + 1) * 1024)
            ppif_t = pp_pool.tile([128, 1024], bf16, tag="ppif")
            nc.sync.dma_start(ppif_t[:], ppif_h[:, csl])
            ppo_t = pp_pool.tile([64, 1024], bf16, tag="ppo")
            nc.sync.dma_start(ppo_t[:], ppo_h[:, csl])

            mf2 = sc.tile([128, 1024], f32, tag="mf")
            nc.vector.tensor_mul(mf2[:], ppif_t[:], C2[:, csl])       # W_cif*C
            nc.vector.tensor_add(mf2[:], mf2[:], p0[:])               # + conv_if
            if dbg_t == t:
                nc.sync.dma_start(dbg_h[:, csl], mf2[:])
            G = sc.tile([128, 1024], f32, tag="g")
            nc.scalar.activation(G[:], mf2[:],
                                 mybir.ActivationFunctionType.Sigmoid,
                                 bias=b_if)                            # [i_g ; f_g]
            tc_t = mosc.tile([64, 1024], f32, tag="tt")
            nc.scalar.activation(tc_t[:], p1[0:64, :],
                                 mybir.ActivationFunctionType.Tanh,
                                 bias=b_c)                             # tanh(c_c)
            vw = sc.tile([128, 1024], f32, tag="vw")
            nc.vector.tensor_mul(vw[0:64, :], G[0:64, :], tc_t[:])    # i_g*tc
            nc.vector.tensor_mul(vw[64:128, :], G[64:128, :], C2[64:128, csl])  # f_g*C
            # two-SBUF-input ops must share a base partition (walrus
            # verifier), so hop f_g*C down to base 0 with a 1-input copy
            w0 = mosc.tile([64, 1024], f32, tag="tt")
            nc.gpsimd.tensor_copy(w0[:], vw[64:128, :])
            nc.vector.tensor_add(C2[0:64, csl], vw[0:64, :], w0[:])   # Cn
            nc.gpsimd.tensor_copy(C2[64:128, csl], C2[0:64, csl])     # dup Cn
            th_t = mosc.tile([64, 1024], f32, tag="tt")
            nc.scalar.activation(th_t[:], C2[0:64, csl],
                                 mybir.ActivationFunctionType.Tanh,
                                 bias=b_zero)                          # tanh(Cn)
            mo = mosc.tile([64, 1024], f32, tag="mo")
            nc.vector.tensor_mul(mo[:], ppo_t[:], C2[0:64, csl])      # W_co*Cn
            nc.vector.tensor_add(mo[:], mo[:], p1[64:128, :])         # + conv_o
            nc.scalar.activation(mo[:], mo[:],
                                 mybir.ActivationFunctionType.Sigmoid,
                                 bias=b_o)                             # o_g in place
            hf = mosc.tile([64, 1024], f32, tag="hf")
            nc.vector.tensor_mul(hf[:], mo[:], th_t[:])               # Hn
            # write-backs: padded bf16 H (both halves) + fp32 output
            hf3 = hf[:].rearrange("p (r c) -> p r c", c=32)
            hrow = (d + 1) * 34 + 1
            nc.scalar.activation(hh3[0:64, hrow:hrow + 32, 1:33], hf3,
                                 mybir.ActivationFunctionType.Copy)
            # upper half holds the one-column-left-shifted copy
            nc.vector.tensor_copy(hh3[64:128, hrow:hrow + 32, 0:32], hf3)
            nc.sync.dma_start(y_h[:, t * nsp + d * 1024: t * nsp + (d + 1) * 1024],
                              hf[:])

        for t in range(t_steps):
            # ---- rebuild X im2col for this step (27 shifted DMA copies) ----
            for j, (kd, kh, kw) in enumerate(TAPS):
                oj = kd * plane + kh * PW + kw
                ln = min(xi_cols, npad - QOFF - oj)
                nc.sync.dma_start(
                    XI[4 * j:4 * j + 4, 0:ln],
                    xpad_h[0:4, t, QOFF + oj: QOFF + oj + ln],
                )
            # Chunk d's conv reads H planes d-1..d+1 of the *previous* step,
            # but emit_el(d) overwrites plane d in place.  Emitting el(d-1)
            # after mm(d) makes Tile's WAR deps order every read of plane
            # d-1 before its overwrite (one-chunk software pipeline).
            prev = None
            for d in range(d_depth):
                cur = emit_mm(t, d)
                if prev is not None:
                    emit_el(t, d - 1, *prev)
                prev = cur
            emit_el(t, d_depth - 1, *prev)

    nc.finalize()
    return nc


# ---------------------------------------------------------------------------
# host-side input prep

def prep_weights(Wc, b, W_ci, W_cf, W_co):
    Wc = np.asarray(Wc, np.float32)
    wh = np.zeros((128, N_WSLOT, 256), np.float32)
    for s, (kd, kh) in enumerate(KDKH):
        # fused slot: lower rows = tap kw=-1, upper rows = tap kw=0
        wh[0:64, s, :] = Wc[:, 4:68, kd + 1, kh + 1, 0].T
        wh[64:128, s, :] = Wc[:, 4:68, kd + 1, kh + 1, 1].T
        # single slot: tap kw=+1, upper rows zero
        wh[0:64, 9 + s, :] = Wc[:, 4:68, kd + 1, kh + 1, 2].T
    for j, (kd, kh, kw) in enumerate(TAPS):
        for c in range(C_IN):
            wh[4 * j + c, 18, :] = Wc[:, c, kd + 1, kh + 1, kw + 1]
    whwx = wh.reshape(128, N_WSLOT * 256).astype(ml_dtypes.bfloat16)

    bias = np.zeros((128, 4), np.float32)
    b = np.asarray(b, np.float32)
    bias[:, 0] = b[0:128]                      # i ; f
    bias[0:64, 1] = b[128:192]                 # c
    bias[0:64, 2] = b[192:256]                 # o

    ppif = np.concatenate([
        np.asarray(W_ci, np.float32).reshape(64, NSP),
        np.asarray(W_cf, np.float32).reshape(64, NSP),
    ], axis=0).astype(ml_dtypes.bfloat16)
    ppo = np.asarray(W_co, np.float32).reshape(64, NSP).astype(ml_dtypes.bfloat16)
    return whwx, bias, ppif, ppo


def prep_x(Xb):
    """[C_IN, T, D, H, W] fp32 -> padded bf16 [C_IN, T, NPAD]."""
    xp = np.zeros((C_IN, T, PD, PH, PW), np.float32)
    xp[:, :, 1:1 + D, 1:1 + HS, 1:1 + WS] = Xb
    return xp.reshape(C_IN, T, NPAD).astype(ml_dtypes.bfloat16)


_NC_CACHE = {}
_LAST_RESULTS = {}


def _get_nc():
    if "nc" not in _NC_CACHE:
        _NC_CACHE["nc"] = build_nc()
    return _NC_CACHE["nc"]


def kernel(X, Wc, b, W_ci, W_cf, W_co):
    import os
    from concourse.bass_utils import run_bass_kernel_spmd

    X = np.asarray(X, np.float32)
    whwx, bias, ppif, ppo = prep_weights(Wc, b, W_ci, W_cf, W_co)
    in_maps = []
    for bi in range(B):
        in_maps.append({
            "xpad": prep_x(X[bi]),
            "whwx": whwx,
            "bias": bias,
            "ppif": ppif,
            "ppo": ppo,
        })
    nc = _get_nc()
    trace = os.environ.get("TRACE_BASS", "0") == "1"
    res = run_bass_kernel_spmd(nc, in_maps, core_ids=list(range(B)), trace=trace)
    _LAST_RESULTS["br"] = res
    out = np.stack([
        np.asarray(res.results[bi]["y"]).reshape(C_OUT, T, D, HS, WS)
        for bi in range(B)
    ], axis=0)
    return out.astype(np.float32)

